# revision 6
# baseline (speedup 1.0000x reference)
"""Trainium2 Bass kernel for nn_Detr3DCrossAttention (DETR3D cross attention).

Sharding: queries are padded 900->1024 and split across 8 NeuronCores (128
queries per core, query = SBUF partition). Each core holds the full
multi-level feature maps as host-built bf16 "row-pair" gather tables
tab[r] = [feat_row(y,x) | feat_row(y+1,x)], so a single 2KB dma_gather
descriptor (elem_size=1024 bf16, elem_step=512) fetches the whole 2x2
bilinear patch for one (query, cam, level) sample. Indices are computed
on-device first and wrapped into the gather engine's 16-partition layout
with a selection matmul (no DRAM bounce); gathers are issued biggest-first
so the descriptor generator feeds the DMA engines continuously. The 4 patch
quadrants are combined per member in bf16 on the vector engine and members
are accumulated exactly in f32 PSUM via identity matmuls. No collectives;
host concatenates the 8 outputs.
"""
import os
import numpy as np
import ml_dtypes

import concourse.bass as bass
import concourse.mybir as mybir
import concourse.tile as tile
from concourse import bacc
from concourse.bass import AP
from concourse.masks import make_identity
from concourse import library_config
from concourse.bass_utils import run_bass_kernel_spmd

dt = mybir.dt
Alu = mybir.AluOpType
Act = mybir.ActivationFunctionType
Ax = mybir.AxisListType

# ---- problem constants (hardcoded per spec) ----
PC_RANGE = (-51.2, -51.2, -5.0, 51.2, 51.2, 3.0)
IMG_H, IMG_W = 928, 1600
EPS = 1e-5
LN_EPS = 1e-5
B, Q, D, N, L = 1, 900, 256, 6, 4
LVL_HW = [(116, 200), (58, 100), (29, 50), (15, 25)]
QPAD = 1024
NCORES = 8
QC = QPAD // NCORES  # 128 queries per core
NMEMB = 24

# gather groups, issued in this order (biggest first so SWDGE descriptor
# generation streams into the DMA engines without idle gaps).
# (name, member_lo, member_hi, rows); member id m = lev*6 + n.
# levels 2+3 share one table (10950 rows, still int16-addressable).
GGROUPS = [
    ("l23", 12, 24, 6 * 29 * 50 + 6 * 15 * 25),
    ("l1a", 6, 11, 5 * 58 * 100),
    ("l1b", 11, 12, 1 * 58 * 100),
    ("l0a", 0, 1, 116 * 200), ("l0b", 1, 2, 116 * 200),
    ("l0c", 2, 3, 116 * 200), ("l0d", 3, 4, 116 * 200),
    ("l0e", 4, 5, 116 * 200), ("l0f", 5, 6, 116 * 200),
]

# member-local row base within its group table
MEMBER_BASE = np.zeros((NMEMB,), np.float32)
for _n in range(6):
    MEMBER_BASE[0 * 6 + _n] = 0.0                      # level0: own table
    MEMBER_BASE[1 * 6 + _n] = (_n % 5) * 58 * 100 if _n < 5 else 0.0
    MEMBER_BASE[2 * 6 + _n] = _n * 29 * 50             # l23 table: lev2 first
    MEMBER_BASE[3 * 6 + _n] = 6 * 29 * 50 + _n * 15 * 25

# shared-constants blob column layout [128, .] f32; "hot" part loads first
_HOT_COLS = [
    ("l2i", 72),        # [k(4), ax(3), n(6)]
    ("lvlc", 36),       # [lev(4), c(9)]: Ws Hs W+1 H+1 W-1 H-1 W-2 H-2 W
    ("base", 24),       # member-local row base, m = lev*6+n
    ("gmask", 8),       # [g] : 1 iff p//16 == g
    ("selr", 128),      # [o] : 1 iff p%16 == o%16
]
_WGT_COLS = [
    ("wattn", 48),      # [c(2), 24]
    ("battn", 24),
    ("wout", 512),      # [c(2), 256]
    ("bout", 256),
    ("wpe2", 512),      # [c(2), 256]
    ("bpe1", 256), ("gpe1", 256), ("bepe1", 256),
    ("bpe2", 256), ("gpe2", 256), ("bepe2", 256),
]
_HOT_OFF = {}
_o = 0
for _nm, _w in _HOT_COLS:
    _HOT_OFF[_nm] = (_o, _o + _w)
    _o += _w
NHOT = _o
_WGT_OFF = {}
_o = 0
for _nm, _w in _WGT_COLS:
    _WGT_OFF[_nm] = (_o, _o + _w)
    _o += _w
NWGT = _o

NPCB = 2 * 256 + 3  # per-core blob: qT(2*128), qpT(2*128), rp_q(3)


def _build_program():
    nc = bacc.Bacc("TRN2", target_bir_lowering=False, debug=False)

    tabs = {}
    for name, mlo, mhi, rows in GGROUPS:
        tabs[name] = nc.dram_tensor(f"tab_{name}", [rows, 2 * D], dt.bfloat16,
                                    kind="ExternalInput")

    hot_d = nc.dram_tensor("hot", [128, NHOT], dt.float32,
                           kind="ExternalInput")
    wgt_d = nc.dram_tensor("wgt", [128, NWGT], dt.float32,
                           kind="ExternalInput")
    pcb_d = nc.dram_tensor("pcb", [128, NPCB], dt.float32,
                           kind="ExternalInput")
    rpT_d = nc.dram_tensor("rpT", [3, QC], dt.float32, kind="ExternalInput")
    wpe1_d = nc.dram_tensor("wpe1", [3, D], dt.float32, kind="ExternalInput")
    out_d = nc.dram_tensor("out", [QC, D], dt.float32, kind="ExternalOutput")

    dbg = os.environ.get("K_DEBUG") == "1"
    if dbg:
        dbg_idx = nc.dram_tensor("dbg_idx", [QC, NMEMB], dt.float32,
                                 kind="ExternalOutput")
        dbg_wrap = nc.dram_tensor("dbg_wrap", [128, NMEMB * 8], dt.int16,
                                  kind="ExternalOutput")
        dbg_wb = nc.dram_tensor("dbg_wb", [QC, 4, 4, 6], dt.float32,
                                kind="ExternalOutput")
        dbg_acc = nc.dram_tensor("dbg_acc", [QC, D], dt.float32,
                                 kind="ExternalOutput")

    F32 = dt.float32
    BF16 = dt.bfloat16

    with tile.TileContext(nc) as tc:
        with tc.tile_pool(name="sb", bufs=1) as sb, \
             tc.tile_pool(name="gbig", bufs=1) as gbig, \
             tc.tile_pool(name="gpool", bufs=3) as gpool, \
             tc.tile_pool(name="tpool", bufs=4) as tpool, \
             tc.tile_pool(name="ps", bufs=1, space="PSUM") as ps, \
             tc.tile_pool(name="pstr", bufs=1, space="PSUM") as pstr:

            V = nc.vector
            S = nc.scalar
            T = nc.tensor
            G = nc.gpsimd

            # ---------------- phase A: loads (hot first) ----------------
            hot = sb.tile([128, NHOT], F32, name="hot", tag="hot")
            nc.sync.dma_start(hot[:], hot_d[:])
            pcb = sb.tile([128, NPCB], F32, name="pcb", tag="pcb")
            nc.sync.dma_start(pcb[:], pcb_d[:])
            wgt = sb.tile([128, NWGT], F32, name="wgt", tag="wgt")
            nc.sync.dma_start(wgt[:], wgt_d[:])
            rpT = sb.tile([3, QC], F32, name="rpT", tag="rpT")
            nc.sync.dma_start(rpT[:], rpT_d[:])
            wpe1 = sb.tile([3, D], F32, name="wpe1", tag="wpe1")
            nc.sync.dma_start(wpe1[:], wpe1_d[:])

            G.load_library(library_config.mlp)

            def hv(nm):
                a, b2 = _HOT_OFF[nm]
                return hot[:, a:b2]

            def wv(nm):
                a, b2 = _WGT_OFF[nm]
                return wgt[:, a:b2]

            l2i = hv("l2i").rearrange("p (k a n) -> p k a n", k=4, a=3, n=6)
            lvlc = hv("lvlc").rearrange("p (l c) -> p l c", l=4, c=9)
            basev = hv("base").rearrange("p (l n) -> p l n", l=4, n=6)
            gmask = hv("gmask")
            selr = hv("selr")

            def bc(ap, shape):
                return ap.to_broadcast(shape)

            def lc(k):
                return bc(lvlc[:, :, k:k + 1], [128, 4, 6])

            def ttile(name, shape, dtype=F32):
                return sb.tile(shape, dtype, name=name, tag=name)

            sh = [128, 4, 6]

            # ---------------- phase B: index path (critical) ----------------
            rp_q = pcb[:, 512:515]
            rw = ttile("rw", [128, 3])
            pr = PC_RANGE
            for k in range(3):
                V.tensor_scalar(out=rw[:, k:k + 1], in0=rp_q[:, k:k + 1],
                                scalar1=float(pr[3 + k] - pr[k]),
                                scalar2=float(pr[k]), op0=Alu.mult, op1=Alu.add)

            cam3 = ttile("cam3", [128, 3, 6])
            V.tensor_scalar(out=cam3[:], in0=l2i[:, 0, :, :],
                            scalar1=rw[:, 0:1], scalar2=None, op0=Alu.mult)
            V.scalar_tensor_tensor(out=cam3[:], in0=l2i[:, 1, :, :],
                                   scalar=rw[:, 1:2], in1=cam3[:],
                                   op0=Alu.mult, op1=Alu.add)
            V.scalar_tensor_tensor(out=cam3[:], in0=l2i[:, 2, :, :],
                                   scalar=rw[:, 2:3], in1=cam3[:],
                                   op0=Alu.mult, op1=Alu.add)
            V.tensor_tensor(out=cam3[:], in0=cam3[:], in1=l2i[:, 3, :, :],
                            op=Alu.add)

            zc = ttile("zc", [128, 6])
            V.tensor_scalar(out=zc[:], in0=cam3[:, 2, :], scalar1=EPS,
                            scalar2=None, op0=Alu.max)
            rz = ttile("rz", [128, 6])
            V.reciprocal(rz[:], zc[:])
            xr = ttile("xr", [128, 6])
            V.tensor_tensor(out=xr[:], in0=cam3[:, 0, :], in1=rz[:], op=Alu.mult)
            yr = ttile("yr", [128, 6])
            V.tensor_tensor(out=yr[:], in0=cam3[:, 1, :], in1=rz[:], op=Alu.mult)

            pxc = ttile("pxc", sh)
            V.tensor_tensor(out=pxc[:], in0=bc(xr[:].unsqueeze(1), sh),
                            in1=lc(0), op=Alu.mult)
            V.tensor_scalar(out=pxc[:], in0=pxc[:], scalar1=-0.5, scalar2=-2.0,
                            op0=Alu.add, op1=Alu.max)
            V.tensor_tensor(out=pxc[:], in0=pxc[:], in1=lc(2), op=Alu.min)
            pyc = ttile("pyc", sh)
            V.tensor_tensor(out=pyc[:], in0=bc(yr[:].unsqueeze(1), sh),
                            in1=lc(1), op=Alu.mult)
            V.tensor_scalar(out=pyc[:], in0=pyc[:], scalar1=-0.5, scalar2=-2.0,
                            op0=Alu.add, op1=Alu.max)
            V.tensor_tensor(out=pyc[:], in0=pyc[:], in1=lc(3), op=Alu.min)

            def floor_of(pc, name):
                ii = sb.tile(sh, dt.int32, name=f"{name}_i", tag=f"{name}_i")
                V.tensor_copy(ii[:], pc[:])
                ff = ttile(f"{name}_f", sh)
                V.tensor_copy(ff[:], ii[:])
                dg = ttile(f"{name}_d", sh)
                V.tensor_tensor(out=dg[:], in0=ff[:], in1=pc[:], op=Alu.is_gt)
                f0 = ttile(f"{name}_0", sh)
                V.tensor_tensor(out=f0[:], in0=ff[:], in1=dg[:], op=Alu.subtract)
                return f0

            x0 = floor_of(pxc, "fx")
            y0 = floor_of(pyc, "fy")

            xs = ttile("xs", sh)
            V.tensor_scalar(out=xs[:], in0=x0[:], scalar1=0.0, scalar2=None,
                            op0=Alu.max)
            V.tensor_tensor(out=xs[:], in0=xs[:], in1=lc(6), op=Alu.min)
            ys = ttile("ys", sh)
            V.tensor_scalar(out=ys[:], in0=y0[:], scalar1=0.0, scalar2=None,
                            op0=Alu.max)
            V.tensor_tensor(out=ys[:], in0=ys[:], in1=lc(7), op=Alu.min)

            idxf = ttile("idxf", sh)
            V.tensor_tensor(out=idxf[:], in0=ys[:], in1=lc(8), op=Alu.mult)
            V.tensor_tensor(out=idxf[:], in0=idxf[:], in1=xs[:], op=Alu.add)
            V.tensor_tensor(out=idxf[:], in0=idxf[:], in1=basev, op=Alu.add)

            # wrap into the 16-partition gather layout via selection matmul:
            # wrap[16a+r, 8m+g] = idx[16g+r, m]
            rhs2 = ttile("rhs2", [128, 8, NMEMB])
            V.tensor_tensor(
                out=rhs2[:],
                in0=bc(idxf[:].rearrange("p l n -> p (l n)").unsqueeze(1),
                       [128, 8, NMEMB]),
                in1=bc(gmask.unsqueeze(2), [128, 8, NMEMB]), op=Alu.mult)
            shuf_ps = ps.tile([128, 8 * NMEMB], F32, name="shuf_ps",
                              tag="shuf_ps")
            T.matmul(out=shuf_ps[:],
                     lhsT=selr, rhs=rhs2[:].rearrange("p g m -> p (g m)"),
                     start=True, stop=True)
            wrap16 = sb.tile([128, NMEMB * 8], dt.int16, name="wrap16",
                             tag="wrap16")
            V.tensor_copy(
                wrap16[:].rearrange("p (m g) -> p m g", m=NMEMB, g=8),
                shuf_ps[:].rearrange("p (g m) -> p m g", g=8, m=NMEMB))

            # ---------------- phase C: gathers (big groups first) -----------
            gouts = {}
            for name, mlo, mhi, rows in GGROUPS:
                nm = mhi - mlo
                nidx = nm * QC
                pool = gbig if name == "l23" else gpool
                pad = [128, 12, 1024] if name == "l23" else [128, 5, 1024]
                gout = pool.tile([128, nm, 1024], BF16, name=f"g_{name}",
                                 tag="gbig" if name == "l23" else "gout",
                                 padded_shape=pad)
                gouts[name] = gout
                if os.environ.get("K_NOGATHER") == "1":
                    V.memset(gout[:], 0.25)
                else:
                    G.dma_gather(
                        out_ap=gout[:],
                        in_ap=AP(tabs[name], 0,
                                 [[2 * D, rows - 1], [1, 4 * D]]),
                        idxs_ap=wrap16[:, mlo * 8:mhi * 8],
                        num_idxs=nidx,
                        num_idxs_reg=nidx,
                        elem_size=4 * D,
                        elem_step=2 * D,
                        single_packet=False,
                    )

            # ---------------- phase D: attention + quadrant weights ---------
            msk = ttile("msk", [128, 6])
            mt = ttile("mt", [128, 6])
            V.tensor_scalar(out=msk[:], in0=cam3[:, 2, :], scalar1=EPS,
                            scalar2=None, op0=Alu.is_gt)
            V.tensor_scalar(out=mt[:], in0=xr[:], scalar1=0.0, scalar2=None,
                            op0=Alu.is_gt)
            V.tensor_tensor(out=msk[:], in0=msk[:], in1=mt[:], op=Alu.mult)
            V.tensor_scalar(out=mt[:], in0=xr[:], scalar1=float(IMG_W),
                            scalar2=None, op0=Alu.is_lt)
            V.tensor_tensor(out=msk[:], in0=msk[:], in1=mt[:], op=Alu.mult)
            V.tensor_scalar(out=mt[:], in0=yr[:], scalar1=0.0, scalar2=None,
                            op0=Alu.is_gt)
            V.tensor_tensor(out=msk[:], in0=msk[:], in1=mt[:], op=Alu.mult)
            V.tensor_scalar(out=mt[:], in0=yr[:], scalar1=float(IMG_H),
                            scalar2=None, op0=Alu.is_lt)
            V.tensor_tensor(out=msk[:], in0=msk[:], in1=mt[:], op=Alu.mult)

            qT = pcb[:, 0:256].rearrange("p (c q) -> p c q", c=2)
            qpT = pcb[:, 256:512].rearrange("p (c q) -> p c q", c=2)
            qsT = ttile("qsT", [128, 2, QC])
            V.tensor_tensor(out=qsT[:], in0=qT, in1=qpT, op=Alu.add)
            wattn = wv("wattn").rearrange("p (c a) -> p c a", c=2)
            aw_ps = ps.tile([QC, 24], F32, name="aw_ps", tag="aw_ps")
            for c in range(2):
                T.matmul(out=aw_ps[:], lhsT=qsT[:, c, :], rhs=wattn[:, c, :],
                         start=(c == 0), stop=(c == 1))
            awl = ttile("awl", [QC, 24])
            V.tensor_tensor(out=awl[:], in0=aw_ps[:], in1=wv("battn"),
                            op=Alu.add)
            aw = ttile("aw", [QC, 24])
            S.activation(aw[:], awl[:], Act.Sigmoid)

            maw = ttile("maw", sh)
            V.tensor_tensor(out=maw[:], in0=bc(msk[:].unsqueeze(1), sh),
                            in1=aw[:].rearrange("p (n l) -> p l n", n=6, l=4),
                            op=Alu.mult)

            fx = ttile("fx_w", sh)
            V.tensor_tensor(out=fx[:], in0=pxc[:], in1=x0[:], op=Alu.subtract)
            fy = ttile("fy_w", sh)
            V.tensor_tensor(out=fy[:], in0=pyc[:], in1=y0[:], op=Alu.subtract)

            def colrow_w(t0, frac, ss, hi_v, hi_r, name):
                vl = ttile(f"{name}_vl", sh)
                V.tensor_scalar(out=vl[:], in0=t0[:], scalar1=0.0, scalar2=None,
                                op0=Alu.is_ge)
                tv = ttile(f"{name}_tv", sh)
                V.tensor_tensor(out=tv[:], in0=t0[:], in1=hi_v, op=Alu.is_le)
                V.tensor_tensor(out=vl[:], in0=vl[:], in1=tv[:], op=Alu.mult)
                vr = ttile(f"{name}_vr", sh)
                V.tensor_scalar(out=vr[:], in0=t0[:], scalar1=-1.0, scalar2=None,
                                op0=Alu.is_ge)
                V.tensor_tensor(out=tv[:], in0=t0[:], in1=hi_r, op=Alu.is_le)
                V.tensor_tensor(out=vr[:], in0=vr[:], in1=tv[:], op=Alu.mult)
                wl = ttile(f"{name}_wl", sh)
                V.tensor_scalar(out=wl[:], in0=frac[:], scalar1=-1.0,
                                scalar2=1.0, op0=Alu.mult, op1=Alu.add)
                V.tensor_tensor(out=wl[:], in0=wl[:], in1=vl[:], op=Alu.mult)
                wr = ttile(f"{name}_wr", sh)
                V.tensor_tensor(out=wr[:], in0=frac[:], in1=vr[:], op=Alu.mult)
                sd = ttile(f"{name}_s", sh)
                V.tensor_tensor(out=sd[:], in0=t0[:], in1=ss[:], op=Alu.subtract)
                e0 = ttile(f"{name}_e0", sh)
                V.tensor_scalar(out=e0[:], in0=sd[:], scalar1=0.0, scalar2=None,
                                op0=Alu.is_equal)
                em = ttile(f"{name}_em", sh)
                V.tensor_scalar(out=em[:], in0=sd[:], scalar1=0.0, scalar2=None,
                                op0=Alu.is_lt)
                ep = ttile(f"{name}_ep", sh)
                V.tensor_scalar(out=ep[:], in0=sd[:], scalar1=0.0, scalar2=None,
                                op0=Alu.is_gt)
                w0 = ttile(f"{name}_w0", sh)
                V.tensor_tensor(out=w0[:], in0=wl[:], in1=e0[:], op=Alu.mult)
                V.tensor_tensor(out=tv[:], in0=wr[:], in1=em[:], op=Alu.mult)
                V.tensor_tensor(out=w0[:], in0=w0[:], in1=tv[:], op=Alu.add)
                w1 = ttile(f"{name}_w1", sh)
                V.tensor_tensor(out=w1[:], in0=wr[:], in1=e0[:], op=Alu.mult)
                V.tensor_tensor(out=tv[:], in0=wl[:], in1=ep[:], op=Alu.mult)
                V.tensor_tensor(out=w1[:], in0=w1[:], in1=tv[:], op=Alu.add)
                return w0, w1

            wc0x, wc1x = colrow_w(x0, fx, xs, lc(4), lc(6), "cx")
            wr0y, wr1y = colrow_w(y0, fy, ys, lc(5), lc(7), "cy")

            a0 = ttile("a0", sh)
            V.tensor_tensor(out=a0[:], in0=wc0x[:], in1=maw[:], op=Alu.mult)
            a1 = ttile("a1", sh)
            V.tensor_tensor(out=a1[:], in0=wc1x[:], in1=maw[:], op=Alu.mult)
            # quadrant block order in the gathered 2KB window:
            # blk0=(ys,xs) blk1=(ys+1,xs) blk2=(ys,xs+1) blk3=(ys+1,xs+1)
            wb = [ttile(f"wb{b}", sh) for b in range(4)]
            V.tensor_tensor(out=wb[0][:], in0=a0[:], in1=wr0y[:], op=Alu.mult)
            V.tensor_tensor(out=wb[1][:], in0=a0[:], in1=wr1y[:], op=Alu.mult)
            V.tensor_tensor(out=wb[2][:], in0=a1[:], in1=wr0y[:], op=Alu.mult)
            V.tensor_tensor(out=wb[3][:], in0=a1[:], in1=wr1y[:], op=Alu.mult)

            identb = sb.tile([128, 128], BF16, name="identb", tag="identb")
            identf = sb.tile([128, 128], F32, name="identf", tag="identf")
            make_identity(nc, identf[:])
            V.tensor_copy(identb[:], identf[:])

            # ---------------- phase E: position-encoder MLP ----------------
            x_cl = ttile("x_cl", [3, QC])
            V.tensor_scalar(out=x_cl[:], in0=rpT[:], scalar1=0.0, scalar2=1.0,
                            op0=Alu.max, op1=Alu.min)
            x1 = ttile("x1", [3, QC])
            V.tensor_scalar(out=x1[:], in0=x_cl[:], scalar1=EPS, scalar2=None,
                            op0=Alu.max)
            x2 = ttile("x2", [3, QC])
            V.tensor_scalar(out=x2[:], in0=x_cl[:], scalar1=-1.0, scalar2=1.0,
                            op0=Alu.mult, op1=Alu.add)
            V.tensor_scalar(out=x2[:], in0=x2[:], scalar1=EPS, scalar2=None,
                            op0=Alu.max)
            rx2 = ttile("rx2", [3, QC])
            V.reciprocal(rx2[:], x2[:])
            ratio = ttile("ratio", [3, QC])
            V.tensor_tensor(out=ratio[:], in0=x1[:], in1=rx2[:], op=Alu.mult)
            isig = ttile("isig", [3, QC])
            S.activation(isig[:], ratio[:], Act.Ln)

            def layernorm(x_sb, g_ap, be_ap, name):
                mu = ttile(f"{name}_mu", [QC, 1])
                V.tensor_reduce(out=mu[:], in_=x_sb[:], axis=Ax.X, op=Alu.add)
                V.tensor_scalar(out=mu[:], in0=mu[:], scalar1=1.0 / D,
                                scalar2=None, op0=Alu.mult)
                xc = ttile(f"{name}_xc", [QC, D])
                V.tensor_scalar(out=xc[:], in0=x_sb[:], scalar1=mu[:, 0:1],
                                scalar2=None, op0=Alu.subtract)
                sq = ttile(f"{name}_sq", [QC, D])
                var = ttile(f"{name}_var", [QC, 1])
                V.tensor_tensor(out=sq[:], in0=xc[:], in1=xc[:], op=Alu.mult)
                V.tensor_reduce(out=var[:], in_=sq[:], axis=Ax.X, op=Alu.add)
                V.tensor_scalar(out=var[:], in0=var[:], scalar1=1.0 / D,
                                scalar2=LN_EPS, op0=Alu.mult, op1=Alu.add)
                sd = ttile(f"{name}_sd", [QC, 1])
                S.activation(sd[:], var[:], Act.Sqrt)
                rs = ttile(f"{name}_rs", [QC, 1])
                V.reciprocal(rs[:], sd[:])
                V.tensor_scalar(out=xc[:], in0=xc[:], scalar1=rs[:, 0:1],
                                scalar2=None, op0=Alu.mult)
                V.tensor_tensor(out=xc[:], in0=xc[:], in1=g_ap, op=Alu.mult)
                V.tensor_tensor(out=xc[:], in0=xc[:], in1=be_ap, op=Alu.add)
                return xc

            h1_ps = ps.tile([QC, D], F32, name="h1_ps", tag="h1_ps")
            T.matmul(out=h1_ps[:], lhsT=isig[:], rhs=wpe1[:], start=True,
                     stop=True)
            h1 = ttile("h1", [QC, D])
            V.tensor_tensor(out=h1[:], in0=h1_ps[:], in1=wv("bpe1"), op=Alu.add)
            h1n = layernorm(h1, wv("gpe1"), wv("bepe1"), "ln1")
            h1r = ttile("h1r", [QC, D])
            S.activation(h1r[:], h1n[:], Act.Relu)

            h1T = ttile("h1T", [128, 2, QC])
            for c in range(2):
                trp = pstr.tile([128, 128], F32, name="trp", tag="trp")
                T.transpose(out=trp[:], in_=h1r[:, c * 128:(c + 1) * 128],
                            identity=identf[:])
                V.tensor_copy(h1T[:, c, :], trp[:])

            wpe2 = wv("wpe2").rearrange("p (c d) -> p c d", c=2)
            h2_ps = ps.tile([QC, D], F32, name="h2_ps", tag="h2_ps")
            for c in range(2):
                T.matmul(out=h2_ps[:], lhsT=h1T[:, c, :], rhs=wpe2[:, c, :],
                         start=(c == 0), stop=(c == 1))
            h2 = ttile("h2", [QC, D])
            V.tensor_tensor(out=h2[:], in0=h2_ps[:], in1=wv("bpe2"), op=Alu.add)
            h2n = layernorm(h2, wv("gpe2"), wv("bepe2"), "ln2")
            h2r = ttile("h2r", [QC, D])
            S.activation(h2r[:], h2n[:], Act.Relu)

            # ---------------- phase F: weighted accumulate ------------------
            # per member: bf16 quadrant combine on DVE, then an identity
            # matmul accumulates members exactly into f32 PSUM.
            acc_ps = ps.tile([QC, D], F32, name="acc_ps", tag="acc_ps")
            n_memb_done = 0
            for name, mlo, mhi, rows in GGROUPS:
                gout = gouts[name]
                for mloc in range(mhi - mlo):
                    m = mlo + mloc
                    lev, n_cam = divmod(m, 6)
                    tmp = tpool.tile([128, 256], BF16, name=f"t{m}", tag="tmp")
                    V.tensor_scalar(out=tmp[:], in0=gout[:, mloc, 0:256],
                                    scalar1=wb[0][:, lev, n_cam:n_cam + 1],
                                    scalar2=None, op0=Alu.mult)
                    for b2 in range(1, 4):
                        V.scalar_tensor_tensor(
                            out=tmp[:],
                            in0=gout[:, mloc, 256 * b2:256 * (b2 + 1)],
                            scalar=wb[b2][:, lev, n_cam:n_cam + 1],
                            in1=tmp[:], op0=Alu.mult, op1=Alu.add)
                    T.matmul(out=acc_ps[:], lhsT=identb[:], rhs=tmp[:],
                             start=(n_memb_done == 0),
                             stop=(n_memb_done == NMEMB - 1))
                    n_memb_done += 1

            # ---------------- phase G: output projection --------------------
            acc = ttile("acc", [QC, D])
            V.tensor_copy(acc[:], acc_ps[:])
            accT = ttile("accT", [128, 2, QC])
            for c in range(2):
                trp2 = pstr.tile([128, 128], F32, name="trp2", tag="trp")
                T.transpose(out=trp2[:], in_=acc[:, c * 128:(c + 1) * 128],
                            identity=identf[:])
                V.tensor_copy(accT[:, c, :], trp2[:])
            wout = wv("wout").rearrange("p (c d) -> p c d", c=2)
            out_ps = ps.tile([QC, D], F32, name="out_ps", tag="out_ps")
            for c in range(2):
                T.matmul(out=out_ps[:], lhsT=accT[:, c, :], rhs=wout[:, c, :],
                         start=(c == 0), stop=(c == 1))
            o1 = ttile("o1", [QC, D])
            V.tensor_tensor(out=o1[:], in0=out_ps[:], in1=wv("bout"),
                            op=Alu.add)
            V.tensor_tensor(out=o1[:], in0=o1[:], in1=h2r[:], op=Alu.add)
            nc.sync.dma_start(out_d[:], o1[:])

            if dbg:
                nc.sync.dma_start(dbg_idx[:], idxf[:].rearrange(
                    "p l n -> p (l n)"))
                nc.sync.dma_start(dbg_wrap[:], wrap16[:])
                wbd = ttile("wbd", [QC, 4, 4, 6])
                for b in range(4):
                    V.tensor_copy(wbd[:, b, :, :], wb[b][:])
                nc.sync.dma_start(dbg_wb[:], wbd[:])
                nc.sync.dma_start(dbg_acc[:], acc[:])

    nc.compile()
    return nc


_NC_CACHE = None


def _get_program():
    global _NC_CACHE
    if _NC_CACHE is None:
        _NC_CACHE = _build_program()
    return _NC_CACHE


def _pair_table(feat_slice):
    """[ncam, C, H, W] f32 -> [ncam*H*W, 2C] bf16 row-pair table."""
    t = feat_slice.astype(ml_dtypes.bfloat16)
    a = np.ascontiguousarray(t.transpose(0, 2, 3, 1))   # [ncam, H, W, C]
    b = np.roll(a, -1, axis=1)                          # row y+1 (y=H-1 unused)
    ncam, H, W, C = a.shape
    return np.concatenate([a, b], axis=-1).reshape(ncam * H * W, 2 * C)


def _host_prep(inputs):
    f32 = np.float32
    query = np.asarray(inputs["query"], f32)[0]
    query_pos = np.asarray(inputs["query_pos"], f32)[0]
    rp = np.asarray(inputs["reference_points"], f32)[0]
    l2i = np.asarray(inputs["lidar2img"], f32)[0]
    feats = [np.asarray(inputs[f"feat{i}"], f32)[0] for i in range(4)]

    def padq(x, fill):
        out = np.full((QPAD,) + x.shape[1:], fill, f32)
        out[:Q] = x
        return out

    query_p = padq(query, 0.0)
    qpos_p = padq(query_pos, 0.0)
    rp_p = padq(rp, 0.5)

    shared = {}
    shared["tab_l23"] = np.concatenate(
        [_pair_table(feats[2]), _pair_table(feats[3])], axis=0)
    shared["tab_l1a"] = _pair_table(feats[1][0:5])
    shared["tab_l1b"] = _pair_table(feats[1][5:6])
    for k, nm in enumerate(["l0a", "l0b", "l0c", "l0d", "l0e", "l0f"]):
        shared[f"tab_{nm}"] = _pair_table(feats[0][k:k + 1])

    hot = np.zeros((128, NHOT), f32)

    def hput(nm, arr):
        a, b2 = _HOT_OFF[nm]
        hot[:, a:b2] = arr

    l2i_r = np.broadcast_to(
        l2i.transpose(2, 1, 0)[:, :3, :][None], (128, 4, 3, 6))
    hput("l2i", np.ascontiguousarray(l2i_r).reshape(128, -1))

    lvlc = np.zeros((4, 9), f32)
    for lv, (H, W) in enumerate(LVL_HW):
        lvlc[lv] = [W / IMG_W, H / IMG_H, W + 1.0, H + 1.0, W - 1.0, H - 1.0,
                    W - 2.0, H - 2.0, float(W)]
    hput("lvlc", np.broadcast_to(lvlc.reshape(-1)[None], (128, 36)))
    hput("base", np.broadcast_to(MEMBER_BASE[None], (128, 24)))
    pvec = np.arange(128)
    hput("gmask", (pvec[:, None] // 16 == np.arange(8)[None, :]).astype(f32))
    hput("selr", (pvec[:, None] % 16 == pvec[None, :] % 16).astype(f32))
    shared["hot"] = hot

    wgt = np.zeros((128, NWGT), f32)

    def wput(nm, arr):
        a, b2 = _WGT_OFF[nm]
        wgt[:, a:b2] = arr

    wput("wattn", np.ascontiguousarray(
        np.asarray(inputs["W_attn"], f32).reshape(2, 128, 24)
        .transpose(1, 0, 2)).reshape(128, -1))
    wput("battn", np.broadcast_to(np.asarray(inputs["b_attn"], f32),
                                  (128, 24)))
    wput("wout", np.ascontiguousarray(
        np.asarray(inputs["W_out"], f32).reshape(2, 128, D)
        .transpose(1, 0, 2)).reshape(128, -1))
    wput("bout", np.broadcast_to(np.asarray(inputs["b_out"], f32), (128, D)))
    wput("wpe2", np.ascontiguousarray(
        np.asarray(inputs["W_pe2"], f32).reshape(2, 128, D)
        .transpose(1, 0, 2)).reshape(128, -1))
    for nm, key in [("bpe1", "b_pe1"), ("gpe1", "g_pe1"), ("bepe1", "be_pe1"),
                    ("bpe2", "b_pe2"), ("gpe2", "g_pe2"), ("bepe2", "be_pe2")]:
        wput(nm, np.broadcast_to(np.asarray(inputs[key], f32), (128, D)))
    shared["wgt"] = wgt
    shared["wpe1"] = np.asarray(inputs["W_pe1"], f32)

    in_maps = []
    for cid in range(NCORES):
        q0 = cid * QC
        m = dict(shared)
        pcb = np.zeros((128, NPCB), f32)
        pcb[:, 0:256] = query_p[q0:q0 + QC].T.reshape(2, 128, QC).transpose(
            1, 0, 2).reshape(128, -1)
        pcb[:, 256:512] = qpos_p[q0:q0 + QC].T.reshape(2, 128, QC).transpose(
            1, 0, 2).reshape(128, -1)
        pcb[:, 512:515] = rp_p[q0:q0 + QC]
        m["pcb"] = pcb
        m["rpT"] = np.ascontiguousarray(rp_p[q0:q0 + QC].T)
        in_maps.append(m)
    return in_maps


def kernel(**inputs):
    nc = _get_program()
    in_maps = _host_prep(inputs)
    res = run_bass_kernel_spmd(nc, in_maps, core_ids=list(range(NCORES)))
    outs = [res.results[cid]["out"] for cid in range(NCORES)]
    full = np.concatenate(outs, axis=0)[:Q]
    return full[None].astype(np.float32)


def kernel_traced(**inputs):
    """test.py helper: also returns exec_time_ns from the NTFF profile."""
    nc = _get_program()
    in_maps = _host_prep(inputs)
    res = run_bass_kernel_spmd(nc, in_maps, core_ids=list(range(NCORES)),
                               trace=True)
    outs = [res.results[cid]["out"] for cid in range(NCORES)]
    full = np.concatenate(outs, axis=0)[:Q]
    return full[None].astype(np.float32), res
